# revision 1
# baseline (speedup 1.0000x reference)
"""Trainium2 Bass kernel for nn_ExtractionLayer (v4, 4-band windowed sparsity).

metric[b,v,f] = sum_p amp[b,f,p] * exp(-c*(vol[v]*filt[f] - q[b,p])^2)
  amp = softmax_p(logits[b,f,p]),  c = 0.5/(sigma+0.001)^2

Sharding: data-parallel over batch B=32 -> 4 b's per core on 8 cores.

Transposed layout: chunks (f, vh) put 128 v's (vol-sorted) on PSUM
partitions and selected (b,p) columns on the free axis. A K=12 bf16
matmul per chunk computes S = x^2 - 2qx + q^2 - lnamp/c (softmax amp
folded into the exponent via ln), ACT does E = exp(-c*S) in ~2048-col
group instructions, DVE does the segmented p-sum (one fp16 halving add
in 2x mode + one short reduce per group).

Windowed sparsity: exp(-c*d^2) < 1e-6 once |d| > sqrt(14/c), so each
chunk keeps only Ks = pow2ceil(max_b #{p: q[b,p] in x-window}) p-slots
per b (max over the GLOBAL batch so all 8 SPMD cores share one
schedule; pow2 Ks so equal PSUM slots never straddle a bank).

4 PE bands: chunks rotate tile_position row 0/32/64/96; band i writes
PSUM bank i of the group tile (concurrent row-tile matmuls must never
share a PSUM bank). The stationary/moving tiles hold band data at
partition blocks 0/32/64/96, which cuts per-partition DMA bytes ~2.5x
vs 2 bands -- input DMA then easily outruns the ~1.9us/group pipeline.

ALL small tensors are precomputed on host in fp64; the schedule is
baked per (sigma, selection counts) and cached.
"""

import sys

for _p in ("/opt/trn_rl_repo", "/root/.axon_site/_ro/trn_rl_repo"):
    if _p not in sys.path:
        sys.path.append(_p)

import numpy as np
import ml_dtypes

BF16 = ml_dtypes.bfloat16

B, V, F, P = 32, 256, 128, 64
NCORES = 8
B_LOC = B // NCORES          # 4 batches per core
NCH = 2 * F                  # 256 chunks: (f, vh)
NK = 12                      # matmul contraction rows
BANK = 512                   # psum cols per bank == per band-quarter
THR_LN = 14.0                # keep q with c*(x-q)^2 <= THR_LN at window edge
PAD_PHI = 100.0              # phi for padding columns -> exp(-c*100) == 0

_cache: dict = {}


class Schedule:
    """Data-dependent but core-independent processing plan.

    Chunks sorted by Ks desc, packed into groups of 4*spb slots
    (spb = 512 // (4*Kg) slots per bank, 4 banks). cid == -1 marks a
    dummy pad chunk (all-pad columns, output discarded).
    """

    def __init__(self, Ks_chunk, order):
        self.groups = []        # (Kg, spb, [cids (len 4*spb, -1 pads)])
        i = 0
        while i < NCH:
            Kg = Ks_chunk[order[i]]
            spb = BANK // (4 * Kg)
            if i == 0:
                spb = max(1, spb // 2)   # small first group: early EXP start
            cap = 4 * spb
            cids = order[i:i + cap]
            i += len(cids)
            cids = list(cids) + [-1] * (cap - len(cids))
            self.groups.append((Kg, spb, cids))
        # per-chunk placement
        self.place = {}         # cid -> (gi, band, slot)
        self.xoff = {}          # cid -> xst col offset (128 wide)
        self.woff = {}          # cid -> wmv col offset (4*Kg wide)
        self.rcol = {}          # cid -> R col base (4 wide)
        self.rbase = []
        xslot = 0
        wcur = 0
        racc = 0
        self.xslots = []        # per group: (xoff0, woff0) for DMA ranges
        for gi, (Kg, spb, cids) in enumerate(self.groups):
            self.rbase.append(racc)
            self.xslots.append((xslot * 128, wcur))
            for idx, cid in enumerate(cids):
                band, slot = idx % 4, idx // 4
                key = cid if cid >= 0 else ("pad", gi, idx)
                self.place[key] = (gi, band, slot)
                self.xoff[key] = (xslot + slot) * 128
                self.woff[key] = wcur + slot * 4 * Kg
                self.rcol[key] = racc + band * 4 * spb + slot * 4
            xslot += spb
            wcur += spb * 4 * Kg
            racc += 16 * spb
        self.xtot = xslot * 128
        self.wtot = wcur
        self.rtot = racc
        self.key = (tuple(Ks_chunk), tuple(order))


def _build(minus_c, sched):
    import concourse.tile as tile
    from concourse import bacc, mybir

    fp32 = mybir.dt.float32
    fp16 = mybir.dt.float16
    bf16 = mybir.dt.bfloat16
    AF = mybir.ActivationFunctionType
    OP = mybir.AluOpType
    import concourse.bass as bass

    nc = bacc.Bacc("TRN2", target_bir_lowering=False, debug=False,
                   num_devices=NCORES)

    d_xst = nc.dram_tensor("xst", [108, sched.xtot], bf16,
                           kind="ExternalInput")
    d_wmv = nc.dram_tensor("wmv", [108, sched.wtot], bf16,
                           kind="ExternalInput")
    d_out = nc.dram_tensor("out", [128, sched.rtot], fp32,
                           kind="ExternalOutput")

    ngroups = len(sched.groups)

    with tile.TileContext(nc) as tc:
        with (
            tc.tile_pool(name="const", bufs=1) as cp,
            tc.tile_pool(name="ering", bufs=2) as ep,
            tc.tile_pool(name="e2ring", bufs=2) as ep2,
            tc.tile_pool(name="e3ring", bufs=2) as ep3,
            tc.tile_pool(name="psS", bufs=2, space=bass.MemorySpace.PSUM) as psS,
        ):
            warm = cp.tile([1, 2], fp32, tag="warm")
            nc.vector.memset(warm[:, :], 0.0)
            zb = cp.tile([128, 1], fp32, tag="zb")
            nc.vector.memset(zb[:, :], 0.0)
            nc.scalar.activation(warm[:, 0:1], warm[:, 1:2], AF.Exp,
                                 bias=zb[0:1, 0:1])

            xst = cp.tile([108, sched.xtot], bf16, tag="xst")
            wmv = cp.tile([108, sched.wtot], bf16, tag="wmv")
            R = cp.tile([128, sched.rtot], fp32, tag="R")

            # input pieces by groups: fine first, then coarse
            gsz = [1, 1, 1, 1, 2, 2]
            while sum(gsz) < ngroups:
                gsz.append(min(3, ngroups - sum(gsz)))
            g0 = 0
            for ng in gsz:
                gb = min(g0 + ng, ngroups)
                x0, w0 = sched.xslots[g0]
                if gb < ngroups:
                    x1, w1 = sched.xslots[gb]
                else:
                    x1, w1 = sched.xtot, sched.wtot
                nc.sync.dma_start(xst[:, x0:x1], d_xst.ap()[:, x0:x1])
                nc.gpsimd.dma_start(wmv[:, w0:w1], d_wmv.ap()[:, w0:w1])
                g0 = gb

            ocursor = 0
            for gi in range(ngroups):
                Kg, spb, cids = sched.groups[gi]
                h = spb * 4 * Kg          # cols per bank (== 512 if full)
                sS = psS.tile([128, 4 * BANK], fp32, tag="S", name="sS")
                for idx, cid in enumerate(cids):
                    key = cid if cid >= 0 else ("pad", gi, idx)
                    band, slot = idx % 4, idx // 4
                    r0 = 32 * band
                    xo = sched.xoff[key]
                    wo = sched.woff[key]
                    pc = band * BANK + slot * 4 * Kg
                    nc.tensor.matmul(
                        sS[:, pc:pc + 4 * Kg],
                        xst[r0:r0 + NK, xo:xo + 128],
                        wmv[r0:r0 + NK, wo:wo + 4 * Kg],
                        start=True, stop=True,
                        tile_position=(r0, 0),
                    )
                E = ep.tile([128, 4 * BANK], fp16, tag="E", name="E")
                Sv = sS[:, :].rearrange("p (u x) -> p u x", u=4)[:, :, 0:h]
                Ev = E[:, :].rearrange("p (u x) -> p u x", u=4)[:, :, 0:h]
                nc.scalar.activation(Ev, Sv, AF.Exp, scale=float(minus_c),
                                     bias=zb[:, 0:1])
                # p-sum: one fp16 halving add (2x mode) + one Kg/2 reduce
                nseg = spb * 4
                E4 = (E[:, :].rearrange("p (u y) -> p u y", u=4)
                      [:, :, 0:nseg * Kg]
                      .rearrange("p u (s x) -> p u s x", x=Kg))
                E2 = ep2.tile([128, 2 * BANK], fp16, tag="E2", name="E2")
                E2v = (E2[:, :].rearrange("p (u y) -> p u y", u=4)
                       [:, :, 0:nseg * (Kg // 2)]
                       .rearrange("p u (s x) -> p u s x", x=Kg // 2))
                nc.vector.tensor_tensor(E2v, E4[:, :, :, 0:Kg // 2],
                                        E4[:, :, :, Kg // 2:Kg], OP.add)
                red_in = E2v
                if Kg >= 32:
                    E3 = ep3.tile([128, BANK], fp16, tag="E3", name="E3")
                    E3v = (E3[:, :].rearrange("p (u y) -> p u y", u=4)
                           [:, :, 0:nseg * (Kg // 4)]
                           .rearrange("p u (s x) -> p u s x", x=Kg // 4))
                    nc.vector.tensor_tensor(E3v, E2v[:, :, :, 0:Kg // 4],
                                            E2v[:, :, :, Kg // 4:Kg // 2],
                                            OP.add)
                    red_in = E3v
                nc.vector.tensor_reduce(
                    R[:, sched.rbase[gi]:sched.rbase[gi] + 16 * spb]
                    .rearrange("p (u s) -> p u s", u=4),
                    red_in, mybir.AxisListType.X, OP.add)
                rend = sched.rbase[gi] + 16 * spb
                if gi % 3 == 2 or gi >= ngroups - 4:
                    nc.sync.dma_start(d_out.ap()[:, ocursor:rend],
                                      R[:, ocursor:rend])
                    ocursor = rend

    nc.compile()
    return nc


def _get_nc(minus_c, sched):
    key = (float(minus_c), sched.key)
    if key not in _cache:
        _cache[key] = _build(minus_c, sched)
    return _cache[key]


def _split3(v):
    """3-way bf16 split of an fp64 array: h + m + l ~= v to ~24 bits."""
    h = v.astype(BF16)
    r = v - h.astype(np.float64)
    m = r.astype(BF16)
    r2 = r - m.astype(np.float64)
    l = r2.astype(BF16)
    return h, m, l


def kernel(q2_obs_scaled, amplitude_logits, volumes, filters, sigma,
           _trace=False, _tmpdir=None):
    from concourse.bass_utils import run_bass_kernel_spmd

    sig = float(np.asarray(sigma).reshape(()))
    minus_c = -0.5 / (sig + 0.001) ** 2
    c = -minus_c
    thr = np.sqrt(THR_LN / c)

    q = np.asarray(q2_obs_scaled, np.float64)                    # (B, P)
    lg = np.asarray(amplitude_logits, np.float64).reshape(B, F, P)
    vol = np.asarray(volumes, np.float64).reshape(V)
    fil = np.asarray(filters, np.float64).reshape(F)

    mx = lg.max(axis=2, keepdims=True)
    lnamp = lg - (mx + np.log(np.exp(lg - mx).sum(axis=2, keepdims=True)))

    # ---- schedule: windowed selection, global over the batch ----
    vperm = np.argsort(vol, kind="stable")
    vs = vol[vperm]
    xs = vs[:, None] * fil[None, :]                              # (V, F)
    sel = [None] * NCH                                           # (B, P) bool
    Ks_chunk = [0] * NCH
    for cid in range(NCH):
        f, vh = cid >> 1, cid & 1
        xw = xs[vh * 128:(vh + 1) * 128, f]
        lo, hi = xw.min() - thr, xw.max() + thr
        m = (q >= lo) & (q <= hi)                                # (B, P)
        sel[cid] = m
        # multiple-of-8 K; uniform slots of 4K with spb=floor(512/4K)
        # slots per bank never straddle a PSUM bank
        n = int(m.sum(axis=1).max())
        Ks_chunk[cid] = max(8, -(-n // 8) * 8)
    order = sorted(range(NCH), key=lambda cix: -Ks_chunk[cix])
    sched = Schedule(Ks_chunk, order)
    nc = _get_nc(minus_c, sched)

    # ---- stationary x-side tile (shared by all cores) ----
    xst = np.zeros((108, sched.xtot), dtype=BF16)
    for cid in range(NCH):
        gi, band, slot = sched.place[cid]
        f, vh = cid >> 1, cid & 1
        xw = xs[vh * 128:(vh + 1) * 128, f]                      # (128,)
        x2h, x2m, x2l = _split3(xw * xw)
        xh, xm, xl = _split3(xw)
        ones = np.ones(128, dtype=BF16)
        rows = [x2h, x2m, x2l, xh, xh, xh, xm, xm, xl, ones, ones, ones]
        xo = sched.xoff[cid]
        for r, arr in enumerate(rows):
            xst[32 * band + r, xo:xo + 128] = arr

    # ---- per-core moving q-side tiles ----
    wh_a, wm_a, wl_a = _split3(-2.0 * q)                         # (B, P)
    phi = q[:, None, :] ** 2 - lnamp / c                         # (B, F, P)
    ph_a, pm_a, pl_a = _split3(phi)

    in_maps = []
    for i in range(NCORES):
        wmv = np.zeros((108, sched.wtot), dtype=BF16)
        # every pad/dummy column: exp -> 0
        for gi, (Kg, spb, cids) in enumerate(sched.groups):
            for idx, cid in enumerate(cids):
                band = idx % 4
                r0 = 32 * band
                key = cid if cid >= 0 else ("pad", gi, idx)
                wo = sched.woff[key]
                wmv[r0 + 0, wo:wo + 4 * Kg] = 1.0
                wmv[r0 + 1, wo:wo + 4 * Kg] = 1.0
                wmv[r0 + 2, wo:wo + 4 * Kg] = 1.0
                wmv[r0 + 9, wo:wo + 4 * Kg] = PAD_PHI
                if cid < 0:
                    continue
                f = cid >> 1
                for bl in range(B_LOC):
                    bg = B_LOC * i + bl
                    ps = np.nonzero(sel[cid][bg])[0]
                    n = len(ps)
                    col = wo + bl * Kg
                    wmv[r0 + 3, col:col + n] = wh_a[bg, ps]
                    wmv[r0 + 4, col:col + n] = wm_a[bg, ps]
                    wmv[r0 + 5, col:col + n] = wl_a[bg, ps]
                    wmv[r0 + 6, col:col + n] = wh_a[bg, ps]
                    wmv[r0 + 7, col:col + n] = wm_a[bg, ps]
                    wmv[r0 + 8, col:col + n] = wh_a[bg, ps]
                    wmv[r0 + 9, col:col + n] = ph_a[bg, f, ps]
                    wmv[r0 + 10, col:col + n] = pm_a[bg, f, ps]
                    wmv[r0 + 11, col:col + n] = pl_a[bg, f, ps]
        in_maps.append({"xst": xst, "wmv": wmv})

    kw = {}
    if _trace:
        kw = {"trace": True, "tmpdir": _tmpdir}
    res = run_bass_kernel_spmd(nc, in_maps, core_ids=list(range(NCORES)), **kw)

    # ---- host unpack: R[v'(sorted), rcol[cid]+b] -> out[b, v, f] ----
    vback = vperm.reshape(2, 128)                                # vh, v'
    out = np.empty((B, V, F), dtype=np.float32)
    rc = np.array([sched.rcol[cid] for cid in range(NCH)])       # (NCH,)
    for i in range(NCORES):
        R = res.results[i]["out"]                                # (128, rtot)
        cols = rc[:, None] + np.arange(B_LOC)[None, :]           # (NCH, 4)
        Rg = R[:, cols]                                          # (128,NCH,4)
        for bl in range(B_LOC):
            o = out[B_LOC * i + bl]                              # (V, F)
            g = Rg[:, :, bl].reshape(128, F, 2)                  # v', f, vh
            for vh in range(2):
                o[vback[vh], :] = g[:, :, vh]
    if _trace:
        return out, res
    return out



# revision 2
# speedup vs baseline: 1.3425x; 1.3425x over previous
"""Trainium2 Bass kernel for nn_ExtractionLayer (v5, stacked v-tiles).

metric[b,v,f] = sum_p amp[b,f,p] * exp(-c*(vol[v]*filt[f] - q[b,p])^2)
  amp = softmax_p(logits[b,f,p]),  c = 0.5/(sigma+0.001)^2

Sharding: data-parallel over batch B=32 -> 4 b's per core on 8 cores
(batch->core assignment optimized to balance the shared schedule).

v5 layout: pieces = (f, 32-v tile of vol-sorted v).  4 pieces with
adjacent q-windows (sorted by window center, any f) stack on the 128
PSUM partitions of one matmul chunk-group.  Narrow windows (32-v span
+ 2*thr instead of 128-v span) cut selected columns ~2.5x vs v4.

Per chunk-group, centering x,q by the window center m lets 2-way bf16
splits reach ~17 bits:  S = X^2 - 2XQ + Q^2 - lnamp/c  with X=x-m,
Q=q-m via a K=13 matmul (X2h,X2m | Xh*Wh,Xh*Wm,Xm*Wh | per-piece
one_j*(Pjh+Pjm)).  ACT does E=exp(-c*S) fp16, DVE reduces x=4
segments to fp16 partials, host sums variable-count partials with
np.add.reduceat (so per-(b,chunk) counts need no global max / pow2).

Columns pack densely into 512-col PSUM banks (chunk-groups split at
4-col boundaries across banks; 4 PE bands = 4 banks per group tile,
rotating tile_position row 0/32/64/96).  All small tensors precomputed
host-side in fp64; schedule cached per (sigma, selection pattern).
"""

import sys

for _p in ("/opt/trn_rl_repo", "/root/.axon_site/_ro/trn_rl_repo"):
    if _p not in sys.path:
        sys.path.append(_p)

import hashlib

import numpy as np
import ml_dtypes

BF16 = ml_dtypes.bfloat16

B, V, F, P = 32, 256, 128, 64
NCORES = 8
B_LOC = B // NCORES
T = 32                  # v rows per piece
S = 128 // T            # pieces per chunk-group
NK = 5 + 2 * S          # matmul contraction rows
BANK = 512              # psum cols per bank
GRAN = 4                # reduce segment width
THR_LN = 5.0            # window: c*(x-q)^2 <= THR_LN at the edge
PAD_PHI = 100.0         # pad cols -> exp(-c*100) == 0

_cache: dict = {}


def _split2(v):
    """2-way bf16 split: h + m ~= v to ~17 bits."""
    h = v.astype(BF16)
    m = (v - h.astype(np.float64)).astype(BF16)
    return h.astype(np.float32), m.astype(np.float32)


def _ceil4(n):
    return -(-n // GRAN) * GRAN


class Schedule:
    """Core-independent processing plan (shared NEFF across 8 cores).

    cgs: chunk-groups of S pieces; widths use the worst core's
    sum of ceil4(n_b); packed bin-major into (group, band) banks.
    """

    def __init__(self, cgs, assign):
        # cgs: list of dicts with keys m, fs, ts, sel(32x64 bool), nb(32,)
        # assign: (8,4) int array, batch ids per core
        self.cgs = cgs
        self.assign = assign
        ncg = len(cgs)
        w_b = np.stack([_ceil4(cg["nb"]) for cg in cgs])        # (ncg, 32)
        core_w = w_b[:, assign.reshape(-1)].reshape(ncg, 8, 4).sum(2)
        self.wcg = core_w.max(1)                                 # (ncg,)

        # ---- pack cg columns bin-major into banks of 512 ----
        # mm pieces: (cg, bin, off, lo, hi) covering cg-local [lo,hi)
        self.pieces = []
        bin_i, off = 0, 0
        for ci in range(ncg):
            w = int(self.wcg[ci])
            lo = 0
            while w > 0:
                take = min(w, BANK - off)
                self.pieces.append((ci, bin_i, off, lo, lo + take))
                off += take
                lo += take
                w -= take
                if off == BANK:
                    bin_i, off = bin_i + 1, 0
        if off > 0:
            # extend last piece's matmul to fill the bank (pad cols)
            ci, bi, o, lo, hi = self.pieces[-1]
            self.pieces[-1] = (ci, bi, o, lo, hi + (BANK - off))
            bin_i += 1
        self.nbins = bin_i
        self.ngroups = -(-self.nbins // 4)
        self.wtot = self.nbins * BANK
        self.rtot = self.nbins * (BANK // GRAN)

        # cg-local col -> global wmv col map
        self.gcol = []
        for ci in range(ncg):
            self.gcol.append(np.empty(int(self.wcg[ci]), np.int64))
        for ci, bi, o, lo, hi in self.pieces:
            n = min(hi, int(self.wcg[ci])) - lo
            if n > 0:
                self.gcol[ci][lo:lo + n] = bi * BANK + o + np.arange(n)

        # ---- stationary slots: one per mm piece, bin-major; per-band ----
        self.band_nslot = [0, 0, 0, 0]
        self.mm = [[] for _ in range(self.ngroups)]   # (beta, off, w, bslot, ci)
        self.xdma = []    # (g, beta, dcol0, dcol1, bcol0, bcol1)
        self.xslot_cg = []  # per piece: (ci, dslot)
        dslot = 0
        cur = {}
        for ci, bi, o, lo, hi in self.pieces:
            g, beta = bi // 4, bi % 4
            bslot = self.band_nslot[beta]
            self.band_nslot[beta] += 1
            self.mm[g].append((beta, o, hi - lo, bslot, ci))
            self.xslot_cg.append((ci, dslot))
            key = (g, beta)
            if key not in cur:
                cur[key] = [dslot, dslot + 1, bslot, bslot + 1]
            else:
                cur[key][1] += 1
                cur[key][3] += 1
            dslot += 1
        self.nslots = dslot
        self.xtot = self.nslots * 128
        for (g, beta), (d0, d1, b0, b1) in sorted(cur.items()):
            self.xdma.append((g, beta, d0 * 128, d1 * 128, b0 * 128, b1 * 128))
        for g in range(self.ngroups):
            self.mm[g].sort(key=lambda t: (t[1], t[0]))
        self.nbg = [min(4, self.nbins - 4 * g) for g in range(self.ngroups)]

        # ---- per-core unpack runs (reduceat over partial cols) ----
        # runs: starts (partial col), owner (ci*4+lb or -1), first flag
        self.runs = []
        for core in range(8):
            bs = assign[core]
            ev = []   # (partial_start, partial_len, owner, first)
            for ci, cg in enumerate(cgs):
                pos = 0
                for lb in range(4):
                    n = int(cg["nb"][bs[lb]])
                    if n == 0:
                        continue
                    k = _ceil4(n) // GRAN
                    # cg-local partial indices pos/4 .. pos/4+k
                    first = True
                    p0 = pos // GRAN
                    while k > 0:
                        gc = self.gcol[ci][p0 * GRAN]
                        # run extends while gcols contiguous
                        run = 1
                        while (run < k and
                               self.gcol[ci][(p0 + run) * GRAN] ==
                               gc + run * GRAN):
                            run += 1
                        ev.append((gc // GRAN, run, ci * 4 + lb, first))
                        first = False
                        p0 += run
                        k -= run
                    pos += _ceil4(n)
            ev.sort()
            starts, owners, firsts = [], [], []
            cur_end = 0
            for st, ln, ow, fi in ev:
                if st > cur_end:
                    starts.append(cur_end)
                    owners.append(-1)
                    firsts.append(True)
                starts.append(st)
                owners.append(ow)
                firsts.append(fi)
                cur_end = st + ln
            if cur_end < self.rtot:
                starts.append(cur_end)
                owners.append(-1)
                firsts.append(True)
            self.runs.append((np.array(starts), np.array(owners),
                              np.array(firsts)))

        h = hashlib.md5(repr((NK, THR_LN, self.nbins,
                              tuple(self.wcg.tolist()),
                              tuple(self.pieces))).encode()).hexdigest()
        self.key = h


def _build(minus_c, sched):
    import concourse.tile as tile
    from concourse import bacc, mybir

    fp32 = mybir.dt.float32
    fp16 = mybir.dt.float16
    bf16 = mybir.dt.bfloat16
    AF = mybir.ActivationFunctionType
    OP = mybir.AluOpType
    import concourse.bass as bass

    nc = bacc.Bacc("TRN2", target_bir_lowering=False, debug=False,
                   num_devices=NCORES)

    d_xst = nc.dram_tensor("xst", [NK, sched.xtot], bf16,
                           kind="ExternalInput")
    d_wmv = nc.dram_tensor("wmv", [NK, sched.wtot], bf16,
                           kind="ExternalInput")
    d_out = nc.dram_tensor("out", [128, sched.rtot], fp16,
                           kind="ExternalOutput")

    ngroups = sched.ngroups
    PMAX = 96 + NK

    with tile.TileContext(nc) as tc:
        with (
            tc.tile_pool(name="const", bufs=1) as cp,
            tc.tile_pool(name="ering", bufs=3) as ep,
            tc.tile_pool(name="psS", bufs=2, space=bass.MemorySpace.PSUM) as psS,
        ):
            warm = cp.tile([1, 2], fp32, tag="warm")
            nc.vector.memset(warm[:, :], 0.0)
            zb = cp.tile([128, 1], fp32, tag="zb")
            nc.vector.memset(zb[:, :], 0.0)
            nc.scalar.activation(warm[:, 0:1], warm[:, 1:2], AF.Exp,
                                 bias=zb[0:1, 0:1])

            xst = cp.tile([PMAX, max(sched.band_nslot) * 128], bf16,
                          tag="xst")
            wmv = cp.tile([PMAX, ngroups * BANK], bf16, tag="wmv")
            R = cp.tile([128, sched.rtot], fp16, tag="R")

            # input DMA: xst pieces on sync queue, wmv on gpsimd queue
            xd = sorted(sched.xdma)
            for g, beta, d0, d1, b0, b1 in xd:
                nc.sync.dma_start(xst[32 * beta:32 * beta + NK, b0:b1],
                                  d_xst.ap()[:, d0:d1])
            for g in range(ngroups):
                for beta in range(sched.nbg[g]):
                    bi = 4 * g + beta
                    nc.gpsimd.dma_start(
                        wmv[32 * beta:32 * beta + NK,
                            g * BANK:(g + 1) * BANK],
                        d_wmv.ap()[:, bi * BANK:(bi + 1) * BANK])

            ocur = 0
            for g in range(ngroups):
                nbg = sched.nbg[g]
                h = nbg * BANK
                sS = psS.tile([128, 4 * BANK], fp32, tag="S", name="sS")
                for beta, off, w, bslot, ci in sched.mm[g]:
                    r0 = 32 * beta
                    nc.tensor.matmul(
                        sS[:, beta * BANK + off:beta * BANK + off + w],
                        xst[r0:r0 + NK, bslot * 128:(bslot + 1) * 128],
                        wmv[r0:r0 + NK, g * BANK + off:g * BANK + off + w],
                        start=True, stop=True,
                        tile_position=(r0, 0),
                    )
                E = ep.tile([128, 4 * BANK], fp16, tag="E", name="E")
                nc.scalar.activation(E[:, 0:h], sS[:, 0:h], AF.Exp,
                                     scale=float(minus_c), bias=zb[:, 0:1])
                Ev = E[:, 0:h].rearrange("p (s x) -> p s x", x=GRAN)
                rb = g * 4 * (BANK // GRAN)
                with nc.allow_low_precision("fp16 partials; host sums fp32"):
                    nc.vector.tensor_reduce(
                        R[:, rb:rb + h // GRAN], Ev,
                        mybir.AxisListType.X, OP.add)
                rend = rb + h // GRAN
                if g % 2 == 1 or g == ngroups - 1:
                    nc.sync.dma_start(d_out.ap()[:, ocur:rend],
                                      R[:, ocur:rend])
                    ocur = rend

    nc.compile()
    return nc


def _get_nc(minus_c, sched):
    key = (float(minus_c), sched.key)
    if key not in _cache:
        _cache[key] = _build(minus_c, sched)
    return _cache[key]


def _make_schedule(c, q, xs):
    """Pieces -> chunk-groups -> batch assignment -> Schedule."""
    thr = np.sqrt(THR_LN / c)
    pieces = []
    for f in range(F):
        for t in range(V // T):
            xw = xs[t * T:(t + 1) * T, f]
            pieces.append((float(xw.min() + xw.max()) / 2, f, t))
    pieces.sort()
    cgs = []
    for i in range(0, len(pieces), S):
        grp = pieces[i:i + S]
        los, his = [], []
        for _, f, t in grp:
            xw = xs[t * T:(t + 1) * T, f]
            los.append(xw.min())
            his.append(xw.max())
        lo, hi = min(los) - thr, max(his) + thr
        sel = (q >= lo) & (q <= hi)
        nb = sel.sum(axis=1).astype(np.int64)
        if nb.max() == 0:
            continue
        cgs.append({"m": (lo + hi) / 2, "fs": [f for _, f, _ in grp],
                    "ts": [t for _, _, t in grp], "sel": sel, "nb": nb})

    # batch -> core assignment: minimize sum_cg max_core width
    w_b = np.stack([_ceil4(cg["nb"]) for cg in cgs])            # (ncg, 32)
    tot = w_b.sum(0)
    order = np.argsort(-tot, kind="stable")
    assign = [[] for _ in range(8)]
    loads = np.zeros(8)
    for b in order:
        ci = int(np.argmin([loads[i] if len(assign[i]) < 4 else 1e18
                            for i in range(8)]))
        assign[ci].append(int(b))
        loads[ci] += tot[b]
    assign = np.array(assign)

    def obj(a):
        return w_b[:, a.reshape(-1)].reshape(-1, 8, 4).sum(2).max(1).sum()

    rng = np.random.RandomState(0)
    best = obj(assign)
    for _ in range(3000):
        c1, c2 = rng.randint(8), rng.randint(8)
        if c1 == c2:
            continue
        i1, i2 = rng.randint(4), rng.randint(4)
        a2 = assign.copy()
        a2[c1, i1], a2[c2, i2] = assign[c2, i2], assign[c1, i1]
        o2 = obj(a2)
        if o2 < best:
            best, assign = o2, a2
    return Schedule(cgs, assign)


def kernel(q2_obs_scaled, amplitude_logits, volumes, filters, sigma,
           _trace=False, _tmpdir=None):
    from concourse.bass_utils import run_bass_kernel_spmd

    sig = float(np.asarray(sigma).reshape(()))
    minus_c = -0.5 / (sig + 0.001) ** 2
    c = -minus_c

    q = np.asarray(q2_obs_scaled, np.float64)                    # (B, P)
    lg = np.asarray(amplitude_logits, np.float64).reshape(B, F, P)
    vol = np.asarray(volumes, np.float64).reshape(V)
    fil = np.asarray(filters, np.float64).reshape(F)

    mx = lg.max(axis=2, keepdims=True)
    lnamp = lg - (mx + np.log(np.exp(lg - mx).sum(axis=2, keepdims=True)))

    vperm = np.argsort(vol, kind="stable")
    vs = vol[vperm]
    xs = vs[:, None] * fil[None, :]                              # (V, F)

    sched = _make_schedule(c, q, xs)
    nc = _get_nc(minus_c, sched)
    cgs = sched.cgs
    ncg = len(cgs)

    # ---- stationary tile (shared by all cores) ----
    xst = np.zeros((NK, sched.xtot), dtype=BF16)
    ones_j = np.zeros((S, 128), dtype=BF16)
    for j in range(S):
        ones_j[j, j * T:(j + 1) * T] = 1.0
    xrows_cg = {}
    for ci, cg in enumerate(cgs):
        X = np.concatenate([xs[t * T:(t + 1) * T, f] - cg["m"]
                            for f, t in zip(cg["fs"], cg["ts"])])
        X2h, X2m = _split2(X * X)
        Xh, Xm = _split2(X)
        xrows_cg[ci] = (X2h, X2m, Xh, Xm)
    for ci, dslot in sched.xslot_cg:
        X2h, X2m, Xh, Xm = xrows_cg[ci]
        c0 = dslot * 128
        xst[0, c0:c0 + 128] = X2h
        xst[1, c0:c0 + 128] = X2m
        xst[2, c0:c0 + 128] = Xh
        xst[3, c0:c0 + 128] = Xh
        xst[4, c0:c0 + 128] = Xm
        for j in range(S):
            xst[5 + 2 * j, c0:c0 + 128] = ones_j[j]
            xst[6 + 2 * j, c0:c0 + 128] = ones_j[j]

    # ---- per-cg moving data for all 32 batches ----
    cg_data = []
    for ci, cg in enumerate(cgs):
        bi_, pi_ = np.nonzero(cg["sel"])          # b-major, p ascending
        Q = q[bi_, pi_] - cg["m"]
        Wh, Wm = _split2(-2.0 * Q)
        Ps = []
        for f in cg["fs"]:
            Ph, Pm = _split2(Q * Q - lnamp[bi_, f, pi_] / c)
            Ps.append((Ph, Pm))
        off_b = np.zeros(B + 1, np.int64)
        np.cumsum(np.bincount(bi_, minlength=B), out=off_b[1:])
        cg_data.append((Wh, Wm, Ps, off_b))

    # ---- per-core moving tiles ----
    in_maps = []
    for core in range(NCORES):
        wmv = np.zeros((NK, sched.wtot), dtype=BF16)
        wmv[0:2] = 1.0
        wmv[5:NK] = PAD_PHI
        bs = sched.assign[core]
        for ci in range(ncg):
            Wh, Wm, Ps, off_b = cg_data[ci]
            gcol = sched.gcol[ci]
            pos = 0
            for lb in range(4):
                b = int(bs[lb])
                n = int(cgs[ci]["nb"][b])
                if n == 0:
                    continue
                seg = slice(off_b[b], off_b[b] + n)
                gc = gcol[pos:pos + n]
                wmv[2, gc] = Wh[seg]
                wmv[3, gc] = Wm[seg]
                wmv[4, gc] = Wh[seg]
                for j in range(S):
                    wmv[5 + 2 * j, gc] = Ps[j][0][seg]
                    wmv[6 + 2 * j, gc] = Ps[j][1][seg]
                pos += _ceil4(n)
        in_maps.append({"xst": xst, "wmv": wmv})

    kw = {}
    if _trace:
        kw = {"trace": True, "tmpdir": _tmpdir}
    res = run_bass_kernel_spmd(nc, in_maps, core_ids=list(range(NCORES)), **kw)

    # ---- host unpack ----
    # dest flat index per (cg, partition)
    dest = np.empty((ncg, 128), np.int64)
    for ci, cg in enumerate(cgs):
        for j, (f, t) in enumerate(zip(cg["fs"], cg["ts"])):
            dest[ci, j * T:(j + 1) * T] = vperm[t * T:(t + 1) * T] * F + f
    out = np.zeros((B, V * F), dtype=np.float64)
    for core in range(NCORES):
        Rr = np.asarray(res.results[core]["out"], np.float16)
        P32 = Rr.astype(np.float32)
        starts, owners, firsts = sched.runs[core]
        red = np.add.reduceat(P32, starts, axis=1)
        vals = np.zeros((128, ncg * 4), np.float32)
        dm = (owners >= 0) & firsts
        vals[:, owners[dm]] = red[:, dm]
        for r in np.nonzero((owners >= 0) & ~firsts)[0]:
            vals[:, owners[r]] += red[:, r]
        v3 = vals.reshape(128, ncg, 4)
        bs = sched.assign[core]
        for lb in range(4):
            out[bs[lb], dest.reshape(-1)] = v3[:, :, lb].T.reshape(-1)
    out = out.reshape(B, V, F).astype(np.float32)
    if _trace:
        return out, res
    return out
